# revision 3
# baseline (speedup 1.0000x reference)
"""Trainium2 Bass kernel for nn_HardwareOptimizedSpikeProcessor.

Reference semantics (per timestep t):
    acc += (s_t @ (W*mask).T) * 2**scale_exp     # [B, Cout]
    spk  = acc >= 2**threshold_exp
    acc  = acc * (1 - spk)
    out[:, :, t] = spk

Strategy:
  - Data-parallel over batch: 8 cores x 8 samples.
  - The matmul contribution c[t] = s_t @ Wm.T is independent of acc, so it is
    computed as one big bf16 matmul per core ([T*B_loc, Cin] x [Cin, Cout]).
    Spikes are 0/1 and masked weights are integers in [-127, 127], both exact
    in bf16; PSUM accumulates in fp32 (sums < 2^24) => bit-exact.
  - The sequential part is a cheap elementwise scan over T=128 steps on the
    vector engine: add, compare (also the spike output), predicated reset.
  - Layouts: state acc is [cout_lo=128 partitions, (cout_hi=16, b=8)] so each
    scan step is a single full-width [128, 128] DVE op.
"""

import sys

for _p in ("/opt/trn_rl_repo",):
    if _p not in sys.path:
        sys.path.insert(0, _p)

import numpy as np
import ml_dtypes

import concourse.bass as bass
import concourse.mybir as mybir
import concourse.tile as tile
from concourse.bass_utils import run_bass_kernel_spmd

B, CIN, COUT, T = 64, 2048, 2048, 128
NCORES = 8
BLOC = B // NCORES          # samples per core
KC = CIN // 128             # 16 contraction chunks
MC = COUT // 128            # 16 output-channel chunks
TBLK = 64                   # timesteps per pipeline block
NBLK = T // TBLK
NFREE = BLOC * TBLK         # matmul free dim (b, t) = 512

_MAX_WAITS = 1


def _split_excess_waits(nc):
    """This container's walrus build accepts at most one sync-wait per
    instruction; spill extra waits onto same-engine NOPs placed before the
    offending instruction."""
    for f in nc.m.functions:
        for bb in f.blocks:
            new_list = []
            for ins in bb.instructions:
                si = ins.sync_info
                waits = list(si.on_wait) if si is not None and si.on_wait else []
                if len(waits) > _MAX_WAITS:
                    extra, keep = waits[:-_MAX_WAITS], waits[-_MAX_WAITS:]
                    for i in range(0, len(extra), _MAX_WAITS):
                        nop = mybir.InstNoOp(
                            name=f"{ins.name}-waitsplit-{i}", ins=[], outs=[]
                        )
                        nop.engine = ins.engine
                        nop.sync_info = mybir.SyncInfo(
                            on_wait=extra[i : i + _MAX_WAITS], on_update=[]
                        )
                        new_list.append(nop)
                    ins.sync_info = mybir.SyncInfo(
                        on_wait=keep,
                        on_update=list(si.on_update) if si.on_update else [],
                    )
                new_list.append(ins)
            bb.instructions[:] = new_list


def _build(thr: float):
    f32 = mybir.dt.float32
    bf16 = mybir.dt.bfloat16
    nc = bass.Bass()

    # W^T arranged [m, cin_lo, cin_hi, cout_lo] so each m-chunk is contiguous.
    wt_d = nc.dram_tensor("wt", [MC, 128, KC, 128], bf16, kind="ExternalInput")
    # spikes arranged [cin_lo, cin_hi, jblk, b*t]
    spk_d = nc.dram_tensor("spk", [128, KC, NBLK, NFREE], bf16, kind="ExternalInput")
    # 2**scale_exp arranged [cout_lo, cout_hi]
    sc_d = nc.dram_tensor("scale", [128, MC], f32, kind="ExternalInput")
    # spike output, bf16 (0/1 exact), [b, cout, t]
    out_d = nc.dram_tensor("out", [BLOC, COUT, T], mybir.dt.uint8, kind="ExternalOutput")
    out_v = out_d.rearrange(
        "b (ch cl) (j t) -> cl ch b j t", cl=128, j=NBLK
    )  # [128, 16, 8, NBLK, TBLK]

    with tile.TileContext(nc) as tc:
        with (
            tc.tile_pool(name="const", bufs=1) as const,
            tc.tile_pool(name="cpool", bufs=2) as cpool,
            tc.tile_pool(name="opool", bufs=2) as opool,
            tc.tile_pool(name="psum", bufs=4, space="PSUM") as psum,
        ):
            wt_sb = const.tile([128, MC, KC, 128], bf16)
            spk_sb = const.tile([128, KC, NBLK, NFREE], bf16)
            sc_sb = const.tile([128, MC], f32)
            acc = const.tile([128, MC * BLOC], f32)
            zeros = const.tile([128, MC * BLOC], f32)

            nc.sync.dma_start(sc_sb[:], sc_d[:])
            for jj in range(NBLK):
                nc.sync.dma_start(spk_sb[:, :, jj, :], spk_d[:, :, jj, :])
            for m in range(MC):
                nc.sync.dma_start(wt_sb[:, m], wt_d[m])
            nc.vector.memset(acc[:], 0.0)
            nc.vector.memset(zeros[:], 0.0)

            acc3 = acc.rearrange("p (ch b) -> p ch b", b=BLOC)
            zeros3 = zeros.rearrange("p (ch b) -> p ch b", b=BLOC)

            for j in range(NBLK):
                cb = cpool.tile([128, MC, NFREE], f32, tag="cblk")
                for m in range(MC):
                    ps = psum.tile([128, NFREE], f32, tag="ps")
                    for k in range(KC):
                        nc.tensor.matmul(
                            ps,
                            lhsT=wt_sb[:, m, k, :],
                            rhs=spk_sb[:, k, j, :],
                            start=(k == 0),
                            stop=(k == KC - 1),
                        )
                    # evacuate psum -> sbuf with the per-channel 2**scale_exp
                    nc.scalar.activation(
                        cb[:, m, :],
                        ps,
                        mybir.ActivationFunctionType.Copy,
                        scale=sc_sb[:, m : m + 1],
                    )
                cb4 = cb.rearrange("p m (b t) -> p m b t", t=TBLK)
                ob = opool.tile([128, MC, BLOC, TBLK], mybir.dt.uint8, tag="oblk")
                for t in range(TBLK):
                    nc.vector.tensor_tensor(
                        acc3[:], acc3[:], cb4[:, :, :, t], mybir.AluOpType.add
                    )
                    nc.vector.tensor_scalar(
                        ob[:, :, :, t], acc3[:], thr, None, mybir.AluOpType.is_ge
                    )
                    nc.vector.copy_predicated(acc3[:], ob[:, :, :, t], zeros3[:])
                for b in range(BLOC):
                    nc.sync.dma_start(out_v[:, :, b, j, :], ob[:, :, b, :])

    _split_excess_waits(nc)
    return nc


_CACHE = {}


def _get_program(thr: float):
    if thr not in _CACHE:
        _CACHE[thr] = _build(thr)
    return _CACHE[thr]


def _prep_inputs(spikes, weights, mask, scale_exp):
    wm = (weights * mask).astype(np.float32)  # integers <= 127, exact
    # [cout, cin] -> W^T [cin, cout] -> [m, cin_lo, cin_hi, cout_lo]
    wt = (
        np.ascontiguousarray(
            wm.T.reshape(KC, 128, MC, 128).transpose(2, 1, 0, 3)
        ).astype(ml_dtypes.bfloat16)
    )
    scale = np.exp2(scale_exp.astype(np.float64)).astype(np.float32)
    sc = np.ascontiguousarray(scale.reshape(MC, 128).T)  # [cout_lo, cout_hi]
    spk_cores = []
    for i in range(NCORES):
        s = spikes[i * BLOC : (i + 1) * BLOC]  # [b, cin, t]
        # -> [cin_lo, cin_hi, j, b, t] -> [128, KC, NBLK, b*t]
        a = (
            s.transpose(1, 0, 2)
            .reshape(KC, 128, BLOC, NBLK, TBLK)
            .transpose(1, 0, 3, 2, 4)
            .reshape(128, KC, NBLK, NFREE)
        )
        spk_cores.append(np.ascontiguousarray(a).astype(ml_dtypes.bfloat16))
    return wt, sc, spk_cores


def kernel(spikes, weights, mask, scale_exp, threshold_exp, **run_kwargs):
    thr = float(2.0 ** int(np.asarray(threshold_exp)))
    nc = _get_program(thr)
    wt, sc, spk_cores = _prep_inputs(
        np.asarray(spikes, dtype=np.float32),
        np.asarray(weights, dtype=np.float32),
        np.asarray(mask, dtype=np.float32),
        np.asarray(scale_exp),
    )
    in_maps = [
        {"wt": wt, "spk": spk_cores[i], "scale": sc} for i in range(NCORES)
    ]
    res = run_bass_kernel_spmd(
        nc, in_maps, core_ids=list(range(NCORES)), **run_kwargs
    )
    outs = [
        np.asarray(res.results[i]["out"]).astype(np.float32)
        for i in range(NCORES)
    ]
    full = np.concatenate(outs, axis=0)  # [B, Cout, T]
    if run_kwargs:
        return full, res
    return full


# revision 5
# speedup vs baseline: 1.0786x; 1.0786x over previous
"""Trainium2 Bass kernel for nn_HardwareOptimizedSpikeProcessor.

Reference semantics (per timestep t):
    acc += (s_t @ (W*mask).T) * 2**scale_exp     # [B, Cout]
    spk  = acc >= 2**threshold_exp
    acc  = acc * (1 - spk)
    out[:, :, t] = spk

Strategy:
  - Data-parallel over batch: 8 cores x 8 samples.
  - The matmul contribution c[t] = s_t @ Wm.T is independent of acc, so it is
    computed as one big bf16 matmul per core ([T*B_loc, Cin] x [Cin, Cout]).
    Spikes are 0/1 and masked weights are integers in [-127, 127], both exact
    in bf16; PSUM accumulates in fp32 (sums < 2^24) => bit-exact.
  - The sequential part is a cheap elementwise scan over T=128 steps on the
    vector engine: add, compare (also the spike output), predicated reset.
  - Layouts: state acc is [cout_lo=128 partitions, (cout_hi=16, b=8)] so each
    scan step is a single full-width [128, 128] DVE op.
"""

import sys

for _p in ("/opt/trn_rl_repo",):
    if _p not in sys.path:
        sys.path.insert(0, _p)

import numpy as np
import ml_dtypes

import concourse.bass as bass
import concourse.mybir as mybir
import concourse.tile as tile
from concourse.bass_utils import run_bass_kernel_spmd

B, CIN, COUT, T = 64, 2048, 2048, 128
NCORES = 8
BLOC = B // NCORES          # samples per core
KC = CIN // 128             # 16 contraction chunks
MC = COUT // 128            # 16 output-channel chunks
TBLK = 64                   # timesteps per pipeline block
NBLK = T // TBLK
NFREE = BLOC * TBLK         # matmul free dim (b, t) = 512

_MAX_WAITS = 1


def _split_excess_waits(nc):
    """This container's walrus build accepts at most one sync-wait per
    instruction; spill extra waits onto same-engine NOPs placed before the
    offending instruction."""
    for f in nc.m.functions:
        for bb in f.blocks:
            new_list = []
            for ins in bb.instructions:
                si = ins.sync_info
                waits = list(si.on_wait) if si is not None and si.on_wait else []
                if len(waits) > _MAX_WAITS:
                    extra, keep = waits[:-_MAX_WAITS], waits[-_MAX_WAITS:]
                    for i in range(0, len(extra), _MAX_WAITS):
                        nop = mybir.InstNoOp(
                            name=f"{ins.name}-waitsplit-{i}", ins=[], outs=[]
                        )
                        nop.engine = ins.engine
                        nop.sync_info = mybir.SyncInfo(
                            on_wait=extra[i : i + _MAX_WAITS], on_update=[]
                        )
                        new_list.append(nop)
                    ins.sync_info = mybir.SyncInfo(
                        on_wait=keep,
                        on_update=list(si.on_update) if si.on_update else [],
                    )
                new_list.append(ins)
            bb.instructions[:] = new_list


def _build(thr: float):
    f32 = mybir.dt.float32
    bf16 = mybir.dt.bfloat16
    u8 = mybir.dt.uint8
    nc = bass.Bass()

    # time blocks (start, len): first block small so the scan starts early;
    # later blocks keep matmul N large (N = 8*len <= 512).
    blocks = [(0, 32), (32, 48), (80, 48)]
    tmax = max(tb for _, tb in blocks)

    # W^T arranged [m, cin_lo, cin_hi, cout_lo] so each m-chunk is contiguous.
    wt_d = nc.dram_tensor("wt", [MC, 128, KC, 128], bf16, kind="ExternalInput")
    # spikes arranged [cin_lo, cin_hi, b, t]
    spk_d = nc.dram_tensor("spk", [128, KC, BLOC, T], bf16, kind="ExternalInput")
    # 2**scale_exp arranged [cout_lo, cout_hi]
    sc_d = nc.dram_tensor("scale", [128, MC], f32, kind="ExternalInput")
    # spike output, uint8 0/1, [b, cout, t]
    out_d = nc.dram_tensor("out", [BLOC, COUT, T], u8, kind="ExternalOutput")
    out_v = out_d.rearrange("b (ch cl) t -> cl ch b t", cl=128)  # [128,16,8,T]

    with tile.TileContext(nc) as tc:
        with (
            tc.tile_pool(name="const", bufs=1) as const,
            tc.tile_pool(name="cpool", bufs=2) as cpool,
            tc.tile_pool(name="opool", bufs=2) as opool,
            tc.tile_pool(name="psum", bufs=4, space="PSUM") as psum,
        ):
            wt_sb = const.tile([128, MC, KC, 128], bf16)
            spk_sb = const.tile([128, KC, BLOC, T], bf16)
            sc_sb = const.tile([128, MC], f32)
            acc = const.tile([128, 128], f32)
            zeros = const.tile([128, 128], f32)

            # order matters: what the first matmul block needs goes first
            nc.sync.dma_start(sc_sb[:], sc_d[:])
            nc.vector.memset(acc[:], 0.0)
            nc.vector.memset(zeros[:], 0.0)
            t0b, tb0 = blocks[0]
            nc.sync.dma_start(
                spk_sb[:, :, :, t0b : t0b + tb0], spk_d[:, :, :, t0b : t0b + tb0]
            )
            for m in range(MC):
                nc.sync.dma_start(wt_sb[:, m], wt_d[m])
            for t0b, tb0 in blocks[1:]:
                nc.sync.dma_start(
                    spk_sb[:, :, :, t0b : t0b + tb0], spk_d[:, :, :, t0b : t0b + tb0]
                )

            for t0, tb in blocks:
                nfree = BLOC * tb
                # c block, time-major: cb[:, t, m*8+b] so each scan step reads
                # one contiguous [128, 128] slice.
                cb = cpool.tile([128, tmax, 128], f32, tag="cblk")
                for m in range(MC):
                    ps_full = psum.tile([128, BLOC * tmax], f32, tag="ps", name="ps")
                    ps = ps_full[:, :nfree]
                    for k in range(KC):
                        # rhs slice [128, (b, t-range)]: b stride T, t stride 1
                        nc.tensor.matmul(
                            ps,
                            lhsT=wt_sb[:, m, k, :],
                            rhs=spk_sb[:, k, :, t0 : t0 + tb],
                            start=(k == 0),
                            stop=(k == KC - 1),
                        )
                    # evacuate psum -> sbuf (transposing (b,t)->(t,b)) with the
                    # per-channel 2**scale_exp fused in
                    nc.scalar.activation(
                        cb[:, :tb, m * BLOC : (m + 1) * BLOC].rearrange(
                            "p t b -> p b t"
                        ),
                        ps.rearrange("p (b t) -> p b t", b=BLOC),
                        mybir.ActivationFunctionType.Copy,
                        scale=sc_sb[:, m : m + 1],
                    )
                ob = opool.tile([128, tmax, 128], u8, tag="oblk")
                for t in range(tb):
                    nc.vector.tensor_tensor(
                        acc[:], acc[:], cb[:, t, :], mybir.AluOpType.add
                    )
                    nc.vector.tensor_scalar(
                        ob[:, t, :], acc[:], thr, None, mybir.AluOpType.is_ge
                    )
                    nc.vector.copy_predicated(acc[:], ob[:, t, :], zeros[:])
                # rearrange to [m, b, t] (so dram t-runs are contiguous), then out
                ob2 = opool.tile([128, MC, BLOC, tmax], u8, tag="oblk2")
                nc.scalar.copy(
                    ob2[:, :, :, :tb],
                    ob[:, :tb, :].rearrange("p t (m b) -> p m b t", b=BLOC),
                )
                for b in range(BLOC):
                    nc.sync.dma_start(
                        out_v[:, :, b, t0 : t0 + tb], ob2[:, :, b, :tb]
                    )

    _split_excess_waits(nc)
    return nc


_CACHE = {}


def _get_program(thr: float):
    if thr not in _CACHE:
        _CACHE[thr] = _build(thr)
    return _CACHE[thr]


def _prep_inputs(spikes, weights, mask, scale_exp):
    wm = (weights * mask).astype(np.float32)  # integers <= 127, exact
    # [cout, cin] -> W^T [cin, cout] -> [m, cin_lo, cin_hi, cout_lo]
    wt = (
        np.ascontiguousarray(
            wm.T.reshape(KC, 128, MC, 128).transpose(2, 1, 0, 3)
        ).astype(ml_dtypes.bfloat16)
    )
    scale = np.exp2(scale_exp.astype(np.float64)).astype(np.float32)
    sc = np.ascontiguousarray(scale.reshape(MC, 128).T)  # [cout_lo, cout_hi]
    spk_cores = []
    for i in range(NCORES):
        s = spikes[i * BLOC : (i + 1) * BLOC]  # [b, cin, t]
        # -> [cin_lo, cin_hi, b, t]
        a = s.transpose(1, 0, 2).reshape(KC, 128, BLOC, T).transpose(1, 0, 2, 3)
        spk_cores.append(np.ascontiguousarray(a).astype(ml_dtypes.bfloat16))
    return wt, sc, spk_cores


def kernel(spikes, weights, mask, scale_exp, threshold_exp, **run_kwargs):
    thr = float(2.0 ** int(np.asarray(threshold_exp)))
    nc = _get_program(thr)
    wt, sc, spk_cores = _prep_inputs(
        np.asarray(spikes, dtype=np.float32),
        np.asarray(weights, dtype=np.float32),
        np.asarray(mask, dtype=np.float32),
        np.asarray(scale_exp),
    )
    in_maps = [
        {"wt": wt, "spk": spk_cores[i], "scale": sc} for i in range(NCORES)
    ]
    res = run_bass_kernel_spmd(
        nc, in_maps, core_ids=list(range(NCORES)), **run_kwargs
    )
    outs = [
        np.asarray(res.results[i]["out"]).astype(np.float32)
        for i in range(NCORES)
    ]
    full = np.concatenate(outs, axis=0)  # [B, Cout, T]
    if run_kwargs:
        return full, res
    return full
